# revision 28
# baseline (speedup 1.0000x reference)
"""Trainium2 Bass kernel for a Mamba layer (LN -> in_proj -> causal dwconv+SiLU
-> low-rank dt -> selective scan -> gate -> out_proj).

Sharding: 8 cores = batch(4) x d_inner-half(2). Each core processes one batch
row and 256 of the 512 inner channels; out_proj emits partial products summed
per core pair on the host.

Scan layout: channels on partitions, (state-pair, channel-block, time) on the
free dimension. Each hardware scan covers two states n,n+1 as one [128, 4096]
op with four segments; segment resets come from poisoned dt columns (large
positive values make exp(A*dt) = 0 at segment starts). One Act exp per state
serves both channel blocks because the S4D-real init makes A[d,n] independent
of d. B/C rows are DMA-broadcast once per 4 states at half volume; the w/hc
multiplies read them twice through stride-0 AP views. Elementwise multiplies
are split DVE/Pool to balance engine time; the depthwise conv and the D-skip
run on the PE as diagonal-weight matmuls.
"""

import numpy as np

import concourse.bacc as bacc
import concourse.bass as bass
import concourse.mybir as mybir
import concourse.tile as tile
from concourse._compat import axon_active
from concourse.bass_utils import run_bass_kernel_spmd

F32 = mybir.dt.float32
F32R = mybir.dt.float32r
FP16 = mybir.dt.float16
AF = mybir.ActivationFunctionType
OP = mybir.AluOpType

DIM = 256          # model dim
DI = 512           # d_inner
SH = 256           # shard channels per core
NST = 64           # d_state
DTR = 16           # dt_rank
DCONV = 4
L = 1024
B = 4
EPS = 1e-5
P = 128            # partitions
NBLK = SH // P     # 2 channel blocks per core
NUBLK = DI // P    # 4 u blocks (full d_inner, for dbl contraction)
FH = L // 2        # matmul moving-free chunk (<=512)
POISON = 1e30      # dt columns poisoned so exp(A*dt)=0 at segment starts

NPAIR = NST // 2   # 32 state pairs
NGRP = NST // 4    # 16 broadcast groups (4 states per DMA)


def _pool_w(i):    # pair index i in 0..31 -> w-mul on Pool?
    return False   # keep DVE's scan feed free of cross-engine waits


def _pool_hc(i):   # all hc on Pool except every 8th (and the tail, so Pool
    return (i % 8) != 7 and i != 30   # drains before the post phase)


def build_nc(shared_a=True):
    nc = bacc.Bacc(
        "TRN2",
        target_bir_lowering=False,
        debug=not axon_active(),
        num_devices=8,
    )

    xT = nc.dram_tensor("xT", [DIM, L], F32R, kind="ExternalInput")
    CT = nc.dram_tensor("CT", [NST, L], FP16, kind="ExternalInput")
    WinT = nc.dram_tensor("WinT", [DIM, DI + SH], F32R, kind="ExternalInput")
    bias_uz = nc.dram_tensor("bias_uz", [P, 6], F32, kind="ExternalInput")
    WxT = nc.dram_tensor("WxT", [DI, DTR + NST], F32R, kind="ExternalInput")
    WdtT = nc.dram_tensor("WdtT", [DTR, SH], F32R, kind="ExternalInput")
    bdt = nc.dram_tensor("bdt", [P, NBLK], F32, kind="ExternalInput")
    Convdiag = nc.dram_tensor("Convdiag", [P, NUBLK * DCONV * P], F32R,
                              kind="ExternalInput")
    convb = nc.dram_tensor("convb", [P, NUBLK], F32, kind="ExternalInput")
    # shared_a: one A column per state; else per channel-block columns
    acols_w = NST if shared_a else NBLK * NST
    Acols = nc.dram_tensor("Acols", [P, acols_w], F32, kind="ExternalInput")
    Ddiag = nc.dram_tensor("Ddiag", [P, NBLK * P], F32R, kind="ExternalInput")
    WoutT = nc.dram_tensor("WoutT", [SH, DIM], F32R, kind="ExternalInput")
    Ident = nc.dram_tensor("Ident", [P, P], FP16, kind="ExternalInput")
    OnesR = nc.dram_tensor("OnesR", [P, P], F32R, kind="ExternalInput")
    outT = nc.dram_tensor("outT", [DIM, L], F32, kind="ExternalOutput")

    with nc.allow_low_precision("f32r tiles for PE fast mode"), \
            tile.TileContext(nc) as tc:
        with (
            tc.tile_pool(name="persist", bufs=1) as pp,
            tc.tile_pool(name="dram", bufs=1, space="DRAM") as dp,
            tc.tile_pool(name="psY", bufs=1, space="PSUM") as psY,
        ):
            bs_dram = dp.tile([NST, L], FP16, name="bs_dram")
            # ---------- long-lived weights / data ----------
            # (xT/WinT load first inside the pre pool; small consts after)
            ones_r = pp.tile([P, P], F32R, name="ones_r")
            ones_k = ones_r[:, 0:1]
            ones_b = ones_r[0:1, :]
            eps_t = pp.tile([1, 1], F32, name="eps_t")
            i_sb = pp.tile([P, P], FP16, name="ident")
            a_sb = pp.tile([P, acols_w], F32, name="acols")
            dd_sb = pp.tile([P, NBLK * P], F32R, name="ddiag")
            cb_sb = pp.tile([P, NUBLK], F32, name="cb")
            buz_sb = pp.tile([P, 6], F32, name="buz")
            bdt_sb = pp.tile([P, NBLK], F32, name="bdt")
            wdtT_sb = pp.tile([DTR, SH], F32R, name="wdtT")
            woutT_sb = [pp.tile([P, DIM], F32R, name=f"woutT{k}") for k in range(2)]

            # long-lived activations
            us_sb = [pp.tile([P, L], F32R, name=f"us{m}") for m in range(NUBLK)]
            dt_sb = pp.tile([P, NBLK * L], F32, name="dtcat")
            dtu_sb = pp.tile([P, NBLK * L], FP16, name="dtucat")
            # xn/winT persist so the z gate projection can run during the scan
            xn_sb = [pp.tile([P, L], F32R, name=f"xn{k}") for k in range(2)]
            winT_sb = [pp.tile([P, DI + SH], F32R, name=f"winT{k}")
                       for k in range(2)]
            sz_sb = [pp.tile([P, L], FP16, name=f"sz{m}") for m in range(NBLK)]

            # ================= PRE phase =================
            with (
                tc.tile_pool(name="pre", bufs=1) as qp,
                tc.tile_pool(name="prew", bufs=2) as wq,
                tc.tile_pool(name="preps", bufs=2, space="PSUM") as psp,
            ):
                xT_sb = [qp.tile([P, L], F32R, name=f"xTt{k}") for k in range(2)]
                for k in range(2):
                    nc.sync.dma_start(xT_sb[k][:], xT[k * P:(k + 1) * P, :])
                for k in range(2):
                    nc.sync.dma_start(winT_sb[k][:], WinT[k * P:(k + 1) * P, :])
                nc.sync.dma_start(ones_r[:], OnesR[:, :])
                wxT_sb = [qp.tile([P, DTR + NST], F32R, name=f"wxT{k}")
                          for k in range(NUBLK)]
                for k in range(NUBLK):
                    nc.sync.dma_start(wxT_sb[k][:], WxT[k * P:(k + 1) * P, :])
                cd_sb = qp.tile([P, NUBLK * DCONV * P], F32R, name="convdiag")
                nc.sync.dma_start(cd_sb[:], Convdiag[:, :])
                nc.sync.dma_start(buz_sb[:], bias_uz[:, :])
                nc.sync.dma_start(cb_sb[:], convb[:, :])
                nc.sync.dma_start(wdtT_sb[:], WdtT[:, :])
                nc.sync.dma_start(bdt_sb[:], bdt[:, :])
                nc.sync.dma_start(a_sb[:], Acols[:, :])
                nc.sync.dma_start(i_sb[:], Ident[:, :])
                nc.sync.dma_start(dd_sb[:], Ddiag[:, :])
                for k in range(2):
                    nc.sync.dma_start(woutT_sb[k][:], WoutT[k * P:(k + 1) * P, :])
                nc.vector.memset(eps_t[:], EPS)

                # ---- LayerNorm (squares on Pool to keep Act queue short) ----
                sq_sb = [qp.tile([P, L], F32R, name=f"lnsq{k}") for k in range(2)]
                for k in range(2):
                    nc.gpsimd.tensor_mul(sq_sb[k][:], xT_sb[k][:], xT_sb[k][:])

                mu_ps = psp.tile([1, L], F32, name="murow", tag="ps")
                m2_ps = psp.tile([1, L], F32, name="m2row", tag="ps")
                for f in range(2):
                    fs = slice(f * FH, (f + 1) * FH)
                    for k in range(2):
                        nc.tensor.matmul(mu_ps[:, fs], ones_k, xT_sb[k][:, fs],
                                         start=(k == 0), stop=(k == 1))
                    for k in range(2):
                        nc.tensor.matmul(m2_ps[:, fs], ones_k, sq_sb[k][:, fs],
                                         start=(k == 0), stop=(k == 1))
                mu_row = qp.tile([1, L], F32R, name="mu_row")
                nc.scalar.mul(mu_row[:], mu_ps[:], 1.0 / DIM)
                m2_row = wq.tile([1, L], F32, name="m2_row", tag="row", bufs=4)
                nc.scalar.mul(m2_row[:], m2_ps[:], 1.0 / DIM)
                musq = wq.tile([1, L], F32, name="musq", tag="row", bufs=4)
                nc.scalar.square(musq[:], mu_row[:])
                var_row = wq.tile([1, L], F32, name="var_row", tag="row", bufs=4)
                nc.vector.tensor_sub(var_row[:], m2_row[:], musq[:])
                std_row = wq.tile([1, L], F32, name="std_row", tag="row", bufs=4)
                nc.scalar.activation(std_row[:], var_row[:], AF.Sqrt, bias=eps_t[:])
                rstd_row = qp.tile([1, L], F32R, name="rstd_row")
                nc.vector.reciprocal(rstd_row[:], std_row[:])

                mu_bc = psp.tile([P, L], F32, name="mu_bc", tag="ps")
                rstd_bc = psp.tile([P, L], F32, name="rstd_bc", tag="ps")
                for f in range(2):
                    fs = slice(f * FH, (f + 1) * FH)
                    nc.tensor.matmul(mu_bc[:, fs], ones_b, mu_row[:, fs],
                                     start=True, stop=True)
                    nc.tensor.matmul(rstd_bc[:, fs], ones_b, rstd_row[:, fs],
                                     start=True, stop=True)
                for f in range(2):
                    fs = slice(f * FH, (f + 1) * FH)
                    for k in range(2):
                        xc = wq.tile([P, FH], F32, name="lnxc", tag="xc")
                        nc.vector.tensor_sub(xc[:], xT_sb[k][:, fs],
                                             mu_bc[:, fs])
                        nc.vector.tensor_mul(xn_sb[k][:, fs], xc[:],
                                             rstd_bc[:, fs])

                # ---- in_proj (u blocks padded left by 4 zero cols for conv) ----
                PAD = 4
                upre_sb = [qp.tile([P, PAD + L], F32R, name=f"upre{m}")
                           for m in range(NUBLK)]
                for m in range(NUBLK):
                    nc.vector.memset(upre_sb[m][:, 0:PAD], 0.0)
                def in_proj_block(m):
                    ps = psp.tile([P, L], F32, name="mm", tag="ps")
                    for f in range(2):
                        fs = slice(f * FH, (f + 1) * FH)
                        for k in range(2):
                            nc.tensor.matmul(
                                ps[:, fs],
                                winT_sb[k][:, m * P:(m + 1) * P],
                                xn_sb[k][:, fs],
                                start=(k == 0), stop=(k == 1))
                    nc.scalar.activation(upre_sb[m][:, PAD:PAD + L], ps[:],
                                         AF.Identity,
                                         bias=buz_sb[:, m:m + 1])

                for m in range(NUBLK):
                    in_proj_block(m)

                # ---- causal depthwise conv (PE diag matmuls) + SiLU ----
                for m in range(NUBLK):
                    cps = psp.tile([P, L], F32, name="convps", tag="ps")
                    for f in range(2):
                        f0 = f * FH
                        for j in range(DCONV):
                            sh = DCONV - 1 - j   # tap j reads u[t + j - 3]
                            dg = cd_sb[:, (m * DCONV + j) * P:
                                       (m * DCONV + j + 1) * P]
                            lo = PAD + f0 - sh
                            nc.tensor.matmul(
                                cps[:, f0:f0 + FH], dg,
                                upre_sb[m][:, lo:lo + FH],
                                start=(j == 0), stop=(j == DCONV - 1))
                    nc.scalar.activation(us_sb[m][:], cps[:], AF.Silu,
                                         bias=cb_sb[:, m:m + 1])

                # ---- dbl = u @ W_x^T -> dtl [16,L], Bs [64,L] ----
                dtl_ps = psp.tile([DTR, L], F32, name="dtlps", tag="ps")
                bs_ps = psp.tile([NST, L], F32, name="bsps", tag="ps")
                for k in range(NUBLK):      # k outer: accumulate as us[k] lands
                    for f in range(2):
                        fs = slice(f * FH, (f + 1) * FH)
                        nc.tensor.matmul(dtl_ps[:, fs], wxT_sb[k][:, 0:DTR],
                                         us_sb[k][:, fs],
                                         start=(k == 0), stop=(k == NUBLK - 1))
                        nc.tensor.matmul(bs_ps[:, fs],
                                         wxT_sb[k][:, DTR:DTR + NST],
                                         us_sb[k][:, fs],
                                         start=(k == 0), stop=(k == NUBLK - 1))
                dtlT_sb = qp.tile([DTR, L], F32R, name="dtlT")
                nc.scalar.copy(dtlT_sb[:], dtl_ps[:])
                bs_lp = qp.tile([NST, L], FP16, name="bs_lp")
                nc.scalar.copy(bs_lp[:], bs_ps[:])
                nc.sync.dma_start(bs_dram[:, :], bs_lp[:])

                # ---- dt = softplus(dtl @ W_dt^T + b_dt) = ln(1+exp(v)) ----
                for m in range(NBLK):
                    ps = psp.tile([P, L], F32, name="mm", tag="ps")
                    for f in range(2):
                        fs = slice(f * FH, (f + 1) * FH)
                        nc.tensor.matmul(ps[:, fs],
                                         wdtT_sb[:, m * P:(m + 1) * P],
                                         dtlT_sb[:, fs], start=True, stop=True)
                    ev = wq.tile([P, L], F32, name="spev", tag="big")
                    nc.scalar.activation(ev[:], ps[:], AF.Exp,
                                         bias=bdt_sb[:, m:m + 1])
                    nc.scalar.activation(dt_sb[:, m * L:(m + 1) * L], ev[:],
                                         AF.Ln, bias=1.0)

                # ---- dtu = dt * u (Pool), then poison dt segment starts ----
                for m in range(NBLK):
                    nc.gpsimd.tensor_mul(dtu_sb[:, m * L:(m + 1) * L],
                                         dt_sb[:, m * L:(m + 1) * L], us_sb[m][:])
                nc.vector.memset(dt_sb[:, 0:1], POISON)
                nc.vector.memset(dt_sb[:, L:L + 1], POISON)



            # ================= SCAN phase =================
            # Stage-major issue with one-group software pipelining: group g's
            # exps/w-muls/scans are issued together with group g-1's hc-muls
            # and PE accumulations, so no engine queue blocks behind a
            # cross-engine dependency that is not yet ready.
            FREE = 4 * L  # (state-pair n2, block m2, time)
            with (
                tc.tile_pool(name="scn_a", bufs=2) as ap_,
                tc.tile_pool(name="scn_w", bufs=2) as wp_,
                tc.tile_pool(name="scn_h", bufs=4) as hp_,
                tc.tile_pool(name="scn_c", bufs=2) as cp_,
                tc.tile_pool(name="bcb", bufs=2) as bp,
                tc.tile_pool(name="bcc", bufs=3) as cbp,
                tc.tile_pool(name="psZ", bufs=1, space="PSUM") as psz,
            ):
                y_ps = [psY.tile([P, L], F32, name=f"yps{m}", tag=f"yps{m}")
                        for m in range(NBLK)]
                # z gate projection + SiLU here: PE/Act have slack mid-scan
                for m in range(NBLK):
                    zps = psz.tile([P, L], F32, name=f"zmm{m}", tag=f"z{m}")
                    for f in range(2):
                        fs = slice(f * FH, (f + 1) * FH)
                        for k in range(2):
                            nc.tensor.matmul(
                                zps[:, fs],
                                winT_sb[k][:, (NUBLK + m) * P:(NUBLK + m + 1) * P],
                                xn_sb[k][:, fs],
                                start=(k == 0), stop=(k == 1))
                    nc.scalar.activation(sz_sb[m][:], zps[:], AF.Silu,
                                         bias=buz_sb[:, NUBLK + m:NUBLK + m + 1])
                dtu_v = dtu_sb[:].rearrange(
                    "p (m t) -> p m t", m=2).unsqueeze(1).to_broadcast(
                    (P, 2, 2, L))

                def qview(t, q):
                    return t[:, q * 2 * L:(q + 1) * 2 * L].rearrange(
                        "p (n t) -> p n t", n=2).unsqueeze(2).to_broadcast(
                        (P, 2, 2, L))

                h_ts, cbbs = {}, {}
                for g in range(NGRP + 1):
                    if g < NGRP:
                        bb = bp.tile([P, 4 * L], FP16, name="bb", tag="bb")
                        cbb = cbp.tile([P, 4 * L], FP16, name="cbb", tag="cbb")
                        cbbs[g] = cbb
                        nc.sync.dma_start(
                            bb[:], bs_dram[4 * g:4 * g + 4, :].unsqueeze(0)
                            .to_broadcast((P, 4, L)))
                        nc.sync.dma_start(
                            cbb[:], CT[4 * g:4 * g + 4, :].unsqueeze(0)
                            .to_broadcast((P, 4, L)))
                        a_ts, w_ts = [], []
                        for q in range(2):
                            i = 2 * g + q          # pair index 0..31
                            n0 = 2 * i
                            a_t = ap_.tile([P, FREE], F32, name="a_t", tag="a")
                            a_ts.append(a_t)
                            for r in range(2):
                                if shared_a:
                                    scol = a_sb[:, n0 + r:n0 + r + 1]
                                    nc.scalar.activation(
                                        a_t[:, r * 2 * L:(r + 1) * 2 * L],
                                        dt_sb[:], AF.Exp, scale=scol)
                                else:
                                    for m in range(NBLK):
                                        scol = a_sb[:, m * NST + n0 + r:
                                                    m * NST + n0 + r + 1]
                                        o0 = r * 2 * L + m * L
                                        nc.scalar.activation(
                                            a_t[:, o0:o0 + L],
                                            dt_sb[:, m * L:(m + 1) * L],
                                            AF.Exp, scale=scol)
                        for q in range(2):
                            i = 2 * g + q
                            w_t = wp_.tile([P, FREE], FP16, name="w_t", tag="w")
                            w_ts.append(w_t)
                            w_v = w_t[:].rearrange("p (n m t) -> p n m t",
                                                   n=2, m=2)
                            eng_w = nc.gpsimd if _pool_w(i) else nc.vector
                            eng_w.tensor_tensor(w_v, dtu_v, qview(bb, q),
                                                OP.mult)
                        for q in range(2):
                            h_t = hp_.tile([P, FREE], FP16, name="h_t", tag="h")
                            h_ts[(g, q)] = h_t
                            nc.vector.tensor_tensor_scan(
                                h_t[:], a_ts[q][:], w_ts[q][:], 0.0,
                                OP.mult, OP.add)
                    if g > 0:
                        gp = g - 1
                        for q in range(2):
                            i = 2 * gp + q
                            h_t = h_ts.pop((gp, q))
                            hc_t = cp_.tile([P, FREE], FP16, name="hc_t",
                                            tag="hc")
                            h_v = h_t[:].rearrange("p (n m t) -> p n m t",
                                                   n=2, m=2)
                            hc_v = hc_t[:].rearrange("p (n m t) -> p n m t",
                                                     n=2, m=2)
                            eng_c = nc.gpsimd if _pool_hc(i) else nc.vector
                            eng_c.tensor_tensor(hc_v, h_v, qview(cbbs[gp], q),
                                                OP.mult)
                            for r in range(2):
                                for m in range(NBLK):
                                    for f in range(2):
                                        c0 = r * 2 * L + m * L + f * FH
                                        nc.tensor.matmul(
                                            y_ps[m][:, f * FH:(f + 1) * FH],
                                            i_sb[:], hc_t[:, c0:c0 + FH],
                                            start=(i == 0 and r == 0),
                                            stop=False)

            # ================= POST phase =================
            with (
                tc.tile_pool(name="post", bufs=2) as op_,
                tc.tile_pool(name="postps", bufs=2, space="PSUM") as psq,
            ):
                # D-skip on the PE: y += diag(D) @ us  (closes the psum group)
                for m in range(NBLK):
                    for f in range(2):
                        fs = slice(f * FH, (f + 1) * FH)
                        nc.tensor.matmul(
                            y_ps[m][:, fs], dd_sb[:, m * P:(m + 1) * P],
                            us_sb[m][:, fs], start=False, stop=True)
                yg_sb = [op_.tile([P, L], F32R, name=f"yg{m}", tag=f"yg{m}")
                         for m in range(NBLK)]
                for m in range(NBLK):
                    nc.vector.tensor_mul(yg_sb[m][:], y_ps[m][:], sz_sb[m][:])

                for m in range(2):
                    for f in range(2):
                        fs = slice(f * FH, (f + 1) * FH)
                        ps = psq.tile([P, FH], F32, name="omm", tag="ps")
                        for k in range(NBLK):
                            nc.tensor.matmul(
                                ps[:], woutT_sb[k][:, m * P:(m + 1) * P],
                                yg_sb[k][:, fs],
                                start=(k == 0), stop=(k == NBLK - 1))
                        o_sb = op_.tile([P, FH], F32, name="o_sb", tag="o_sb")
                        nc.scalar.copy(o_sb[:], ps[:])
                        nc.sync.dma_start(outT[m * P:(m + 1) * P, fs], o_sb[:])

    nc.finalize()
    return nc


_NC = None
_SHARED_A = None


def _get_nc(shared_a=True):
    global _NC, _SHARED_A
    if _NC is None or _SHARED_A != shared_a:
        _NC = build_nc(shared_a)
        _SHARED_A = shared_a
    return _NC


def make_in_maps(x, C_SA, gamma, beta, W_in, conv_w, conv_b, W_x, W_dt, b_dt,
                 A_log, D, W_out):
    x = np.ascontiguousarray(x, np.float32)
    C_SA = np.ascontiguousarray(C_SA, np.float32)
    A = -np.exp(np.asarray(A_log, np.float32))
    shared_a = bool(np.allclose(A, A[0:1, :], rtol=0, atol=0))
    W_in_eff = np.asarray(W_in, np.float32) * np.asarray(gamma, np.float32)[None, :]
    bias_in = np.asarray(W_in, np.float32) @ np.asarray(beta, np.float32)
    cw = np.asarray(conv_w, np.float32)[:, 0, :]          # [DI, 4]
    cb = np.asarray(conv_b, np.float32)
    W_x = np.asarray(W_x, np.float32)
    W_dt = np.asarray(W_dt, np.float32)
    b_dt = np.asarray(b_dt, np.float32)
    D = np.asarray(D, np.float32)
    W_out = np.asarray(W_out, np.float32)

    ident = np.eye(P, dtype=np.float16)

    def colpack(v, nblk):  # [nblk*128] -> [128, nblk]
        return np.ascontiguousarray(v.reshape(nblk, P).T)

    def diagpack(vs):  # list of [128] -> [128, len*128] block-diagonal cols
        out = np.zeros((P, len(vs) * P), np.float32)
        for b_, v in enumerate(vs):
            out[:, b_ * P:(b_ + 1) * P] = np.diag(v)
        return out

    in_maps = []
    for c in range(8):
        b = c // 2
        sh = c % 2
        perm = np.concatenate([np.arange(sh * SH, (sh + 1) * SH),
                               np.arange((1 - sh) * SH, (2 - sh) * SH)])
        zrows = DI + np.arange(sh * SH, (sh + 1) * SH)
        shard = perm[:SH]
        conv_diags = [np.diag(cw[perm[m * P:(m + 1) * P], j])
                      for m in range(NUBLK) for j in range(DCONV)]
        if shared_a:
            acols = np.ascontiguousarray(
                np.broadcast_to(A[0:1, :], (P, NST)))
        else:
            acols = np.ascontiguousarray(
                A[shard].reshape(NBLK, P, NST).transpose(1, 0, 2).reshape(P, -1))
        in_maps.append({
            "xT": np.ascontiguousarray(x[b].T),
            "CT": np.ascontiguousarray(C_SA[b].T.astype(np.float16)),
            "WinT": np.ascontiguousarray(
                np.concatenate([W_in_eff[perm], W_in_eff[zrows]], 0).T),
            "bias_uz": colpack(np.concatenate([bias_in[perm], bias_in[zrows]]), 6),
            "WxT": np.ascontiguousarray(W_x[:, perm].T),
            "WdtT": np.ascontiguousarray(W_dt[shard].T),
            "bdt": colpack(b_dt[shard], NBLK),
            "Convdiag": np.concatenate(conv_diags, axis=1),
            "convb": colpack(cb[perm], NUBLK),
            "Acols": acols,
            "Ddiag": diagpack([D[shard[m * P:(m + 1) * P]]
                               for m in range(NBLK)]),
            "WoutT": np.ascontiguousarray(W_out[:, shard].T),
            "Ident": ident,
            "OnesR": np.ones((P, P), np.float32),
        })
    return in_maps, shared_a


_RUNNER = None


def _get_runner(shared_a=True):
    """Build (once) a cached jitted 8-core executor mirroring
    bass2jax.run_bass_via_pjrt's shard_map path."""
    global _RUNNER
    if _RUNNER is not None and _RUNNER[-1] == shared_a:
        return _RUNNER
    import jax
    from jax.sharding import Mesh, PartitionSpec
    from jax.experimental.shard_map import shard_map
    import concourse.mybir as mybir_
    from concourse.bass2jax import (
        _bass_exec_p, install_neuronx_cc_hook, partition_id_tensor)

    nc = _get_nc(shared_a)
    install_neuronx_cc_hook()
    n_cores = 8
    partition_name = (nc.partition_id_tensor.name
                      if nc.partition_id_tensor else None)

    in_names, out_names, out_avals = [], [], []
    for alloc in nc.m.functions[0].allocations:
        if not isinstance(alloc, mybir_.MemoryLocationSet):
            continue
        name = alloc.memorylocations[0].name
        if alloc.kind == "ExternalInput":
            if name != partition_name:
                in_names.append(name)
        elif alloc.kind == "ExternalOutput":
            shape = tuple(alloc.tensor_shape)
            dtype = mybir_.dt.np(alloc.dtype)
            out_names.append(name)
            out_avals.append(jax.core.ShapedArray(shape, dtype))
    n_params = len(in_names)
    n_outs = len(out_avals)
    all_names = in_names + out_names
    donate = tuple(range(n_params, n_params + n_outs))

    if partition_name is not None:
        all_names.append(partition_name)

    def _body(*args):
        operands = list(args)
        if partition_name is not None:
            operands.append(partition_id_tensor())
        outs = _bass_exec_p.bind(
            *operands,
            out_avals=tuple(out_avals),
            in_names=tuple(all_names),
            out_names=tuple(out_names),
            lowering_input_output_aliases=(),
            sim_require_finite=True,
            sim_require_nnan=True,
            nc=nc,
        )
        return tuple(outs)

    devices = jax.devices()[:n_cores]
    mesh = Mesh(np.asarray(devices), ("core",))
    in_specs = (PartitionSpec("core"),) * (n_params + n_outs)
    out_specs = (PartitionSpec("core"),) * n_outs
    sharded = jax.jit(
        shard_map(_body, mesh=mesh, in_specs=in_specs, out_specs=out_specs,
                  check_rep=False),
        donate_argnums=donate, keep_unused=True)

    _RUNNER = (nc, sharded, in_names, out_names, out_avals, n_cores, shared_a)
    return _RUNNER


def _execute(in_maps, shared_a=True):
    nc, sharded, in_names, out_names, out_avals, n_cores, _ = \
        _get_runner(shared_a)
    concat_in = [
        np.concatenate([np.asarray(m[name]) for m in in_maps], axis=0)
        for name in in_names
    ]
    concat_zeros = [
        np.zeros((n_cores * a.shape[0], *a.shape[1:]), a.dtype) for a in out_avals
    ]
    out_arrs = sharded(*concat_in, *concat_zeros)
    return [
        {name: np.asarray(out_arrs[i]).reshape(n_cores, *out_avals[i].shape)[c]
         for i, name in enumerate(out_names)}
        for c in range(n_cores)
    ]


def _run(trace=False, **inputs):
    in_maps, shared_a = make_in_maps(**inputs)
    if axon_active():
        results = _execute(in_maps, shared_a)
    else:
        results = run_bass_kernel_spmd(
            _get_nc(shared_a), in_maps, core_ids=list(range(8)),
            trace=trace).results
    outs = [r["outT"] for r in results]
    out = np.stack([(outs[2 * b] + outs[2 * b + 1]).T for b in range(B)])
    return np.ascontiguousarray(out, np.float32), results


def kernel(**inputs):
    out, _ = _run(**inputs)
    return out
